# revision 17
# baseline (speedup 1.0000x reference)
"""Trainium2 Bass kernel for patch attention:
    out = softmax(silu(q) @ silu(k)^T * scale, axis=-1)
with q,k: [B=4, H=16, P=1024, D=128] fp32, scale: [1] fp32.

Sharding: B*H = 64 heads split across 8 NeuronCores, 8 heads each.

Per-core pipeline (per head g), all ACT work in ONE table set
(exp_and_others: tanh + exp), so heads pipeline freely with no
ACT table reloads:
  1. DMA q[g], k[g] as [128, 8, 128] (p-in-tile, p-tile, d) fp32.
  2. PE transpose each [128,128] tile -> PSUM: xT [d=128, p=1024] fp32.
  3. ACT tanh(x/2) (PSUM->SBUF bf16)  [tanh is in the exp table set]
  4. DVE scalar_tensor_tensor: bT = (tanh+1) * xT = 2*silu(x) -> bf16.
     The 2x factors are folded into the softmax scale (scale/4).
  5. Per 128-row p-tile m: two PE matmuls (N=512, bf16) -> scores
     PSUM [128,1024] fp32 (= 4 * silu-scores).
  6. ACT Exp((scale/4)*s) PSUM->SBUF fp32 with accum_out row sums.
  7. DVE reciprocal of sums; DVE tensor_scalar_mul normalizes.
  8. DMA out [128, 1024] fp32 rows to HBM.
"""

import numpy as np

B, H, P, D = 4, 16, 1024, 128
N_CORES = 8
G = (B * H) // N_CORES  # heads per core = 8
PT = P // 128  # p-tiles per head = 8

_cached = {}


def _build_module(mm_dtype_name="bfloat16"):
    import concourse.bass as bass
    import concourse.tile as tile
    from concourse import bacc, mybir
    from concourse.masks import make_identity

    f32 = mybir.dt.float32
    mm_dt = getattr(mybir.dt, mm_dtype_name)
    AF = mybir.ActivationFunctionType

    nc = bacc.Bacc("TRN2", target_bir_lowering=False, debug=False)
    q_d = nc.dram_tensor("q", [G, P, D], f32, kind="ExternalInput")
    k_d = nc.dram_tensor("k", [G, P, D], f32, kind="ExternalInput")
    scale_d = nc.dram_tensor("scale", [1], f32, kind="ExternalInput")
    out_d = nc.dram_tensor("out", [G, P, P], f32, kind="ExternalOutput")

    with tile.TileContext(nc) as tc:
        with (
            tc.tile_pool(name="consts", bufs=1) as consts,
            tc.tile_pool(name="nat", bufs=7) as natp,
            tc.tile_pool(name="th", bufs=4) as thp,
            tc.tile_pool(name="bt", bufs=4) as btp,
            tc.tile_pool(name="exp", bufs=4) as expp,
            tc.tile_pool(name="outs", bufs=6) as outp,
            tc.tile_pool(name="stats", bufs=8) as statp,
            tc.tile_pool(name="ps_t", bufs=2, space="PSUM") as ps_tp,
            tc.tile_pool(name="ps_s", bufs=2, space="PSUM") as ps_sp,
        ):
            identity = consts.tile([128, 128], f32)
            make_identity(nc, identity)
            scale_sb = consts.tile([128, 1], f32)
            nc.gpsimd.dma_start(out=scale_sb, in_=scale_d[:].to_broadcast([128, 1]))
            # bT = 2*silu => scores are 4x; fold the 1/4 into the exp scale
            scale_adj = consts.tile([128, 1], f32)
            nc.vector.tensor_scalar_mul(scale_adj, scale_sb, 0.25)


            def prep_a(g):
                """DMA-in + PE-transposes for head g -> psum xT tiles."""
                out = {}
                for nm, src in (("k", k_d), ("q", q_d)):
                    nat = natp.tile([128, PT, 128], f32, tag="nat", name=f"nat_{nm}{g}")
                    nc.sync.dma_start(
                        out=nat, in_=src[g].rearrange("(t p) d -> p t d", p=128)
                    )
                    ps_t = ps_tp.tile([128, P], f32, tag="ps_t", name=f"psT_{nm}{g}")
                    for t in range(PT):
                        nc.tensor.transpose(
                            ps_t[:, bass.ts(t, 128)], nat[:, t, :], identity
                        )
                    out[nm] = ps_t
                return out

            def prep_b(g, ps):
                """tanh + stt for head g: bT = (tanh(xT/2)+1)*xT = 2*silu(x)^T.
                Emitted AFTER head (g-depth)'s softmax so exps/reduces keep
                priority in the ACT/DVE FIFOs."""
                bts = {}
                for nm in ("k", "q"):
                    ps_t = ps[nm]
                    # tanh(x/2) in the exp_and_others table set
                    th = thp.tile([128, P], mm_dt, tag="th", name=f"th_{nm}{g}")
                    nc.scalar.activation(out=th, in_=ps_t, func=AF.Tanh, scale=0.5)
                    bt = btp.tile([128, P], mm_dt, tag=f"bt_{nm}", name=f"bt_{nm}{g}")
                    nc.vector.scalar_tensor_tensor(
                        out=bt,
                        in0=th,
                        scalar=1.0,
                        in1=ps_t,
                        op0=mybir.AluOpType.add,
                        op1=mybir.AluOpType.mult,
                    )
                    bts[nm] = bt
                return bts["q"], bts["k"]

            # software pipeline, depth 3, engine-queue-aware emission order:
            # per iteration g emit
            #   transposes(g+3) [PE first],
            #   matmul+exp+reduce+norm+dma for head g,
            #   tanh+stt(g+3)   [ACT/DVE after the critical exps/reduces].
            DEPTH = 3
            ready = [prep_b(g, prep_a(g)) for g in range(DEPTH)]
            for g in range(G):
                qbT, kbT = ready.pop(0)
                ps_next = prep_a(g + DEPTH) if g + DEPTH < G else None

                for m in range(PT):
                    ps_s = ps_sp.tile([128, P], f32, tag="ps_s", name=f"psS_{g}_{m}")
                    for h in range(2):
                        nc.tensor.matmul(
                            ps_s[:, bass.ts(h, 512)],
                            qbT[:, bass.ts(m, 128)],
                            kbT[:, bass.ts(h, 512)],
                            start=True,
                            stop=True,
                        )
                    exp_t = expp.tile([128, P], f32, tag="exp", name=f"exp_{g}_{m}")
                    nc.scalar.activation(
                        out=exp_t, in_=ps_s, func=AF.Exp, scale=scale_adj
                    )
                    sum_t = statp.tile([128, 1], f32, tag="sum", name=f"sum_{g}_{m}")
                    nc.vector.tensor_reduce(
                        out=sum_t,
                        in_=exp_t,
                        axis=mybir.AxisListType.X,
                        op=mybir.AluOpType.add,
                    )
                    out_t = outp.tile([128, P], f32, tag="out", name=f"out_{g}_{m}")
                    nc.gpsimd.normalize_recip(out_t, exp_t, sum_t)
                    nc.sync.dma_start(
                        out=out_d[g, bass.ts(m, 128), :], in_=out_t
                    )

                if ps_next is not None:
                    ready.append(prep_b(g + DEPTH, ps_next))

    nc.compile()
    return nc


def _get_nc():
    if "nc" not in _cached:
        _cached["nc"] = _build_module()
    return _cached["nc"]


def kernel(q, k, scale, _trace=False):
    from concourse.bass_utils import run_bass_kernel_spmd

    nc = _get_nc()
    qf = np.ascontiguousarray(q.reshape(B * H, P, D), dtype=np.float32)
    kf = np.ascontiguousarray(k.reshape(B * H, P, D), dtype=np.float32)
    sc = np.ascontiguousarray(scale.reshape(1), dtype=np.float32)
    in_maps = [
        {"q": qf[i * G : (i + 1) * G], "k": kf[i * G : (i + 1) * G], "scale": sc}
        for i in range(N_CORES)
    ]
    res = run_bass_kernel_spmd(
        nc, in_maps, core_ids=list(range(N_CORES)), trace=_trace
    )
    out = np.concatenate([res.results[i]["out"] for i in range(N_CORES)], axis=0)
    if _trace:
        kernel.last_result = res
    return out.reshape(B, H, P, P)


# revision 18
# speedup vs baseline: 1.1097x; 1.1097x over previous
"""Trainium2 Bass kernel for patch attention:
    out = softmax(silu(q) @ silu(k)^T * scale, axis=-1)
with q,k: [B=4, H=16, P=1024, D=128] fp32, scale: [1] fp32.

Sharding: B*H = 64 heads split across 8 NeuronCores, 8 heads each.

Per-core pipeline (per head g), all ACT work in ONE table set
(exp_and_others: tanh + exp), so heads pipeline freely with no
ACT table reloads:
  1. DMA q[g], k[g] as [128, 8, 128] (p-in-tile, p-tile, d) fp32.
  2. PE transpose each [128,128] tile -> PSUM: xT [d=128, p=1024] fp32.
  3. ACT tanh(x/2) (PSUM->SBUF bf16)  [tanh is in the exp table set]
  4. DVE scalar_tensor_tensor: bT = (tanh+1) * xT = 2*silu(x) -> bf16.
     The 2x factors are folded into the softmax scale (scale/4).
  5. Per 128-row p-tile m: two PE matmuls (N=512, bf16) -> scores
     PSUM [128,1024] fp32 (= 4 * silu-scores).
  6. ACT Exp((scale/4)*s) PSUM->SBUF fp32 with accum_out row sums.
  7. DVE reciprocal of sums; DVE tensor_scalar_mul normalizes.
  8. DMA out [128, 1024] fp32 rows to HBM.
"""

import numpy as np

B, H, P, D = 4, 16, 1024, 128
N_CORES = 8
G = (B * H) // N_CORES  # heads per core = 8
PT = P // 128  # p-tiles per head = 8

_cached = {}


def _build_module(mm_dtype_name="bfloat16"):
    import concourse.bass as bass
    import concourse.tile as tile
    from concourse import bacc, mybir
    from concourse.masks import make_identity

    f32 = mybir.dt.float32
    mm_dt = getattr(mybir.dt, mm_dtype_name)
    AF = mybir.ActivationFunctionType

    nc = bacc.Bacc("TRN2", target_bir_lowering=False, debug=False)
    q_d = nc.dram_tensor("q", [G, P, D], f32, kind="ExternalInput")
    k_d = nc.dram_tensor("k", [G, P, D], f32, kind="ExternalInput")
    scale_d = nc.dram_tensor("scale", [1], f32, kind="ExternalInput")
    out_d = nc.dram_tensor("out", [G, P, P], f32, kind="ExternalOutput")

    with tile.TileContext(nc) as tc:
        with (
            tc.tile_pool(name="consts", bufs=1) as consts,
            tc.tile_pool(name="nat", bufs=7) as natp,
            tc.tile_pool(name="th", bufs=4) as thp,
            tc.tile_pool(name="bt", bufs=4) as btp,
            tc.tile_pool(name="exp", bufs=4) as expp,
            tc.tile_pool(name="outs", bufs=6) as outp,
            tc.tile_pool(name="stats", bufs=8) as statp,
            tc.tile_pool(name="ps_t", bufs=2, space="PSUM") as ps_tp,
            tc.tile_pool(name="ps_s", bufs=2, space="PSUM") as ps_sp,
        ):
            identity = consts.tile([128, 128], f32)
            make_identity(nc, identity)
            scale_sb = consts.tile([128, 1], f32)
            nc.gpsimd.dma_start(out=scale_sb, in_=scale_d[:].to_broadcast([128, 1]))
            # bT = 2*silu => scores are 4x; fold the 1/4 into the exp scale
            scale_adj = consts.tile([128, 1], f32)
            nc.vector.tensor_scalar_mul(scale_adj, scale_sb, 0.25)


            def prep_a(g):
                """DMA-in + PE-transposes for head g -> psum xT tiles."""
                out = {}
                for nm, src in (("k", k_d), ("q", q_d)):
                    nat = natp.tile([128, PT, 128], f32, tag="nat", name=f"nat_{nm}{g}")
                    nc.sync.dma_start(
                        out=nat, in_=src[g].rearrange("(t p) d -> p t d", p=128)
                    )
                    ps_t = ps_tp.tile([128, P], f32, tag="ps_t", name=f"psT_{nm}{g}")
                    for t in range(PT):
                        nc.tensor.transpose(
                            ps_t[:, bass.ts(t, 128)], nat[:, t, :], identity
                        )
                    out[nm] = ps_t
                return out

            def prep_b(g, ps):
                """tanh + stt for head g: bT = (tanh(xT/2)+1)*xT = 2*silu(x)^T.
                Emitted AFTER head (g-depth)'s softmax so exps/reduces keep
                priority in the ACT/DVE FIFOs."""
                bts = {}
                for nm in ("k", "q"):
                    ps_t = ps[nm]
                    # tanh(x/2) in the exp_and_others table set
                    th = thp.tile([128, P], mm_dt, tag="th", name=f"th_{nm}{g}")
                    nc.scalar.activation(out=th, in_=ps_t, func=AF.Tanh, scale=0.5)
                    bt = btp.tile([128, P], mm_dt, tag=f"bt_{nm}", name=f"bt_{nm}{g}")
                    nc.vector.scalar_tensor_tensor(
                        out=bt,
                        in0=th,
                        scalar=1.0,
                        in1=ps_t,
                        op0=mybir.AluOpType.add,
                        op1=mybir.AluOpType.mult,
                    )
                    bts[nm] = bt
                return bts["q"], bts["k"]

            # software pipeline, depth 3, engine-queue-aware emission order:
            # per iteration g emit
            #   transposes(g+3) [PE first],
            #   matmul+exp+reduce+norm+dma for head g,
            #   tanh+stt(g+3)   [ACT/DVE after the critical exps/reduces].
            DEPTH = 3
            ready = [prep_b(g, prep_a(g)) for g in range(DEPTH)]
            for g in range(G):
                qbT, kbT = ready.pop(0)
                if g + DEPTH < G:
                    ready.append(prep_b(g + DEPTH, prep_a(g + DEPTH)))

                for m in range(PT):
                    ps_s = ps_sp.tile([128, P], f32, tag="ps_s", name=f"psS_{g}_{m}")
                    for h in range(2):
                        nc.tensor.matmul(
                            ps_s[:, bass.ts(h, 512)],
                            qbT[:, bass.ts(m, 128)],
                            kbT[:, bass.ts(h, 512)],
                            start=True,
                            stop=True,
                        )
                    exp_t = expp.tile([128, P], f32, tag="exp", name=f"exp_{g}_{m}")
                    nc.scalar.activation(
                        out=exp_t, in_=ps_s, func=AF.Exp, scale=scale_adj
                    )
                    sum_t = statp.tile([128, 1], f32, tag="sum", name=f"sum_{g}_{m}")
                    nc.vector.tensor_reduce(
                        out=sum_t,
                        in_=exp_t,
                        axis=mybir.AxisListType.X,
                        op=mybir.AluOpType.add,
                    )
                    out_t = outp.tile([128, P], f32, tag="out", name=f"out_{g}_{m}")
                    nc.gpsimd.normalize_recip(out_t, exp_t, sum_t)
                    nc.sync.dma_start(
                        out=out_d[g, bass.ts(m, 128), :], in_=out_t
                    )

    nc.compile()
    return nc


def _get_nc():
    if "nc" not in _cached:
        _cached["nc"] = _build_module()
    return _cached["nc"]


def kernel(q, k, scale, _trace=False):
    from concourse.bass_utils import run_bass_kernel_spmd

    nc = _get_nc()
    qf = np.ascontiguousarray(q.reshape(B * H, P, D), dtype=np.float32)
    kf = np.ascontiguousarray(k.reshape(B * H, P, D), dtype=np.float32)
    sc = np.ascontiguousarray(scale.reshape(1), dtype=np.float32)
    in_maps = [
        {"q": qf[i * G : (i + 1) * G], "k": kf[i * G : (i + 1) * G], "scale": sc}
        for i in range(N_CORES)
    ]
    res = run_bass_kernel_spmd(
        nc, in_maps, core_ids=list(range(N_CORES)), trace=_trace
    )
    out = np.concatenate([res.results[i]["out"] for i in range(N_CORES)], axis=0)
    if _trace:
        kernel.last_result = res
    return out.reshape(B, H, P, P)
